# revision 5
# baseline (speedup 1.0000x reference)
"""CenterLoss kernel v2 for Trainium2 (8 NeuronCores, data-parallel over N).

loss = sum_{n,c,w} act[n,c,w] * dist[n,c,w],  clipped at 1e-6, where
  dist[n,c,w] = x2[n,w] - 2*xc[n,c,w] + c2[c]

Transposed formulation (per core, NPER=2 n-values):
  W = [-2*c^T | ones | c2]  [C, 66]   (col 64 pairs with x2 row, col 65
                                       with the ones row of xaug)
  y'[n][w, j] = sum_c act[n,c,w] * W[c,j]     (PE: act chunk as lhsT)
  loss_n = sum_{w,j} xt[n][w, j] * y'[n][w, j],  xt = [x; x2; ones]^T

Staging (host, fp8e4m3):
  - acts [NPER*C, W]: natural layout; lhsT slices [80, 128] per w-block.
  - xts  [NPER*128, NBLK*66]: per 128-w-block transposed, block-major.
  - wm [C, 66].
Device: 7 w-blocks per 512-fp32 psum bank (462 used); per 2-bank psum tile
(14 blocks) the elementwise reads strided 3D views [p, 2, 462]:
  - 14 matmuls lhsT=act[:, blk*128:+128], rhs=wm -> pd bank-packed
  - route 'dve': DVE STT (pd + 0) * xt chunk, accum_out -> racc col
  - route 'pool': ACT copy pd(3D) -> SBUF bf16 packed; Pool STT * xt chunk
Tail: racc [128, 20] DMA'd to host; final reduction + clip on host.
"""

import os
import sys

import numpy as np

for _p in ("/opt/trn_rl_repo",):
    if _p not in sys.path and os.path.isdir(_p):
        sys.path.insert(0, _p)

N, D, C, W = 16, 64, 80, 16384
NCORES = 8
NPER = N // NCORES  # 2
J = 66
NBLK = W // 128  # 128
GB = 7  # blocks per psum bank group (462 of 512 fp32)
GW = GB * J  # 462
NGRP = (NBLK + GB - 1) // GB  # 19 (18 full + ragged of 2 blocks)
GPT = 2  # groups per psum tile
NTIL = (NGRP + GPT - 1) // GPT  # 10 (9 full tiles + ragged tile)
NOPS = NPER * NTIL

XT_DT = "float8e4"
ACT_DT = "float8e4"
W_DT = "float8e4"

# per-n chunk routes.  R1 'dve': STT direct from PSUM on DVE.  Chains:
# ACT copies PSUM->SBUF bf16, Pool multiplies by xt (tensor_tensor), and
# the row-sum runs on DVE ('r2d', cheap 4x tensor_scalar) or ACT ('r2a').
_ROUTE_T = {
    0: {1: "split", 2: "r2a", 3: "r2a", 4: "r2d"},
    1: {1: "r2a", 3: "r2d", 5: "r2a", 7: "split"},
}

# act DMA slices per n, in blocks (aligned so no psum tile spans two)
ASLICES = [(0, 14), (14, 70), (70, 128)]
# xt DMA slices per n, in groups; issue order = list order (ragged early,
# a single full tile's data last so the final STT is cheap and prompt)
XSLICES = [(0, 2), (2, 8), (8, 12), (12, 16), (16, 19)]

_CACHE = {}


def _blocks_of_group(g):
    b0 = g * GB
    return list(range(b0, min(b0 + GB, NBLK)))


def _xt_cols(g):
    """[start, end) cols of group g in the unpadded block-major xts."""
    return g * GW, min((g + 1) * GW, NBLK * J)


def _build_bass():
    import concourse.bacc as bacc
    import concourse.tile as tile
    from concourse import mybir

    fp32 = mybir.dt.float32
    bf16 = mybir.dt.bfloat16
    dt_xt = getattr(mybir.dt, XT_DT)
    dt_act = getattr(mybir.dt, ACT_DT)
    dt_w = getattr(mybir.dt, W_DT)
    Alu = mybir.AluOpType

    nc = bacc.Bacc("TRN2", target_bir_lowering=False)

    acts = nc.dram_tensor("acts", [NPER * C, W], dt_act, kind="ExternalInput")
    xts = nc.dram_tensor("xts", [NPER * 128, NBLK * J], dt_xt, kind="ExternalInput")
    wm = nc.dram_tensor("wm", [C, J], dt_w, kind="ExternalInput")
    out = nc.dram_tensor("out", [128, NOPS + 2], fp32, kind="ExternalOutput")

    from contextlib import ExitStack

    with tile.TileContext(nc) as tc, ExitStack() as ctx:
        consts = ctx.enter_context(tc.tile_pool(name="consts", bufs=1))
        dpool = ctx.enter_context(tc.tile_pool(name="dpool", bufs=1))
        spool = ctx.enter_context(tc.tile_pool(name="spool", bufs=6))
        cpool = ctx.enter_context(tc.tile_pool(name="cpool", bufs=6))
        s2pool = ctx.enter_context(tc.tile_pool(name="s2pool", bufs=3))
        rpool = ctx.enter_context(tc.tile_pool(name="rpool", bufs=1))
        pdist = ctx.enter_context(tc.tile_pool(name="pdist", bufs=3, space="PSUM"))
        pchain = ctx.enter_context(tc.tile_pool(name="pchain", bufs=1, space="PSUM"))

        wm_t = consts.tile([C, J], dt_w)
        nc.scalar.dma_start(out=wm_t[:], in_=wm[:, :])

        racc = rpool.tile([128, NOPS + 2], fp32)


        at_sl = {}
        xt_sl = {}

        def _issue_a(ni, si):
            b0, b1 = ASLICES[si]
            t = dpool.tile([C, (b1 - b0) * 128], dt_act, tag=f"a{ni}{si}")
            nc.sync.dma_start(
                out=t[:], in_=acts[ni * C : (ni + 1) * C, b0 * 128 : b1 * 128]
            )
            at_sl[(ni, si)] = t

        def _issue_x(ni, xi):
            g0, g1 = XSLICES[xi]
            c0, _ = _xt_cols(g0)
            c1 = _xt_cols(g1 - 1)[1]
            xt = dpool.tile([128, c1 - c0], dt_xt, tag=f"x{ni}{xi}")
            nc.sync.dma_start(out=xt[:], in_=xts[ni * 128 : (ni + 1) * 128, c0:c1])
            xt_sl[(ni, xi)] = xt

        for ni in range(NPER):
            for si in range(len(ASLICES)):
                _issue_a(ni, si)
                b1 = ASLICES[si][1]
                for xi, (g0, g1) in enumerate(XSLICES):
                    if (ni, xi) in xt_sl:
                        continue
                    if min(g1 * GB, NBLK) > b1:  # xt slice past staged act
                        continue
                    _issue_x(ni, xi)

        def _act_slice(b):
            for si, (b0, b1) in enumerate(ASLICES):
                if b0 <= b < b1:
                    return si, b0
            raise AssertionError(b)

        def _xt_slice(g):
            for xi, (g0, g1) in enumerate(XSLICES):
                if g0 <= g < g1:
                    return xi, g0
            raise AssertionError(g)

        # deferred summers: emit chain sums ~2 chunks later so the ACT/DVE
        # SEQ never head-of-line blocks on a Pool TT still in flight
        pending = []

        def _emit_sum(item):
            route, scr_t, fd2, col = item
            s2 = s2pool.tile([128, GPT * GW], bf16, tag="s2")
            if route == "r2d":
                nc.vector.tensor_scalar(
                    s2[:, 0:fd2],
                    scr_t[:, 0:fd2],
                    1.0,
                    0.0,
                    Alu.mult,
                    op1=Alu.add,
                    accum_out=racc[:, col : col + 1],
                )
            else:
                nc.scalar.activation(
                    out=s2[:, 0:fd2],
                    in_=scr_t[:, 0:fd2],
                    func=mybir.ActivationFunctionType.Copy,
                    accum_out=racc[:, col : col + 1],
                )

        # emit each n's first chain tile before its t0 so the chain's matmuls
        # run first on the PE and the ACT->Pool pipeline starts ASAP
        _ORDER = {0: list(range(NTIL)), 1: list(range(NTIL))}
        _CI = {}
        k = 0
        for _ni in range(NPER):
            for _t in range(NTIL):
                _CI[(_ni, _t)] = k
                k += 1

        for ni in range(NPER):
            for t in _ORDER[ni]:
                ci = _CI[(ni, t)]
                groups = list(range(t * GPT, min((t + 1) * GPT, NGRP)))
                ragged = groups[-1] == NGRP - 1
                route = _ROUTE_T[ni].get(t, "dve")
                pool_for = pdist if (route == "dve" or ragged) else pchain

                def _mms(pd_t, gs, off):
                    for li, g in enumerate(gs):
                        for i, b in enumerate(_blocks_of_group(g)):
                            si, ab0 = _act_slice(b)
                            at = at_sl[(ni, si)]
                            cb = b - ab0
                            nc.tensor.matmul(
                                pd_t[
                                    :,
                                    off + li * 512 + i * J : off + li * 512 + (i + 1) * J,
                                ],
                                at[:, cb * 128 : (cb + 1) * 128],
                                wm_t[:],
                                start=True,
                                stop=True,
                            )

                if route != "split":
                    pd = pool_for.tile([128, GPT * 512], fp32, tag="pd")
                    _mms(pd, groups, 0)
                xi, xg0 = _xt_slice(groups[0])
                xt = xt_sl[(ni, xi)]
                x0 = (groups[0] - xg0) * GW
                if not ragged:
                    # full tile: 2 groups, strided 3D views [128, 2, 462]
                    if route != "split":
                        pdv = pd[:, 0 : GPT * 512].rearrange(
                            "p (g q) -> p g q", g=GPT
                        )[:, :, 0:GW]
                        xtv = xt[:, x0 : x0 + GPT * GW].rearrange(
                            "p (g q) -> p g q", g=GPT
                        )
                    fd2 = GPT * GW
                    if route == "dve":
                        scr = spool.tile([128, GPT * GW], bf16, tag="scr")
                        scv = scr[:, 0:fd2].rearrange("p (g q) -> p g q", g=GPT)
                        nc.vector.scalar_tensor_tensor(
                            out=scv,
                            in0=pdv,
                            scalar=0.0,
                            in1=xtv,
                            op0=Alu.add,
                            op1=Alu.mult,
                            accum_out=racc[:, ci : ci + 1],
                        )
                    elif route == "split":
                        # two independent 1-bank pd tiles: group 1 chains via
                        # pchain (copy can start as soon as its 7 mms land),
                        # group 0 goes DVE-direct via pdist — no co-read
                        # serialization between the two halves
                        pdB = pchain.tile([128, GPT * 512], fp32, tag="pd")
                        _mms(pdB, [groups[1]], 0)
                        cp = cpool.tile([128, GPT * GW], bf16, tag="cp")
                        nc.scalar.copy(out=cp[:, 0:GW], in_=pdB[:, 0:GW])
                        scr2 = spool.tile([128, GPT * GW], bf16, tag="scr")
                        nc.gpsimd.tensor_tensor(
                            out=scr2[:, 0:GW],
                            in0=cp[:, 0:GW],
                            in1=xt[:, x0 + GW : x0 + 2 * GW],
                            op=Alu.mult,
                        )
                        pending.append(("r2d", scr2, GW, NOPS + ni))
                        pdA = pdist.tile([128, GPT * 512], fp32, tag="pd")
                        _mms(pdA, [groups[0]], 0)
                        scr = spool.tile([128, GPT * GW], bf16, tag="scr")
                        nc.vector.scalar_tensor_tensor(
                            out=scr[:, 0:GW],
                            in0=pdA[:, 0:GW],
                            scalar=0.0,
                            in1=xt[:, x0 : x0 + GW],
                            op0=Alu.add,
                            op1=Alu.mult,
                            accum_out=racc[:, ci : ci + 1],
                        )
                    else:
                        cp = cpool.tile([128, GPT * GW], bf16, tag="cp")
                        cpv = cp[:, 0:fd2].rearrange("p (g q) -> p g q", g=GPT)
                        nc.scalar.copy(out=cpv, in_=pdv)
                        scr = spool.tile([128, GPT * GW], bf16, tag="scr")
                        nc.gpsimd.tensor_tensor(
                            out=scr[:, 0:fd2],
                            in0=cp[:, 0:fd2],
                            in1=xt[:, x0 : x0 + fd2],
                            op=Alu.mult,
                        )
                        pending.append((route, scr, fd2, ci))
                else:
                    # ragged tile: single group, 2 blocks = 132 cols, 2D
                    fd = len(_blocks_of_group(groups[0])) * J
                    scr = spool.tile([128, GPT * GW], bf16, tag="scr")
                    nc.vector.scalar_tensor_tensor(
                        out=scr[:, 0:fd],
                        in0=pd[:, 0:fd],
                        scalar=0.0,
                        in1=xt[:, x0 : x0 + fd],
                        op0=Alu.add,
                        op1=Alu.mult,
                        accum_out=racc[:, ci : ci + 1],
                    )
                # drain deferred sums, keeping a few chains in flight
                while len(pending) > 3:
                    _emit_sum(pending.pop(0))
        while pending:
            _emit_sum(pending.pop(0))

        nc.sync.dma_start(out=out[:, :], in_=racc[:])

    nc.compile()
    return nc


def _get_nc():
    if "nc" not in _CACHE:
        _CACHE["nc"] = _build_bass()
    return _CACHE["nc"]


def _np_dt(name):
    import ml_dtypes

    return {
        "float8e4": ml_dtypes.float8_e4m3,
        "bfloat16": ml_dtypes.bfloat16,
        "float32": np.float32,
    }[name]


def _stage(x, c, act):
    """Host-side staging: returns per-core in_maps."""
    xt_np = _np_dt(XT_DT)
    act_np = _np_dt(ACT_DT)
    w_np = _np_dt(W_DT)

    c2 = np.sum(c * c, axis=0, dtype=np.float32)  # [C]
    x2 = np.einsum("ndw,ndw->nw", x, x, dtype=np.float32)  # [N, W]

    wmat = np.empty((C, J), dtype=np.float32)
    wmat[:, 0:D] = -2.0 * c.T
    wmat[:, D] = 1.0  # pairs with xaug row 64 (x2) -> x2[w] term
    wmat[:, D + 1] = c2  # pairs with xaug row 65 (ones) -> c2[c] term
    wmat = np.ascontiguousarray(wmat.astype(w_np))

    xaug = np.empty((N, J, W), dtype=np.float32)
    xaug[:, 0:D, :] = x
    xaug[:, D, :] = x2
    xaug[:, D + 1, :] = 1.0
    # block-major transposed: xts[n, p, b*J+j] = xaug[n, j, b*128+p]
    xt_all = (
        xaug.astype(xt_np)
        .reshape(N, J, NBLK, 128)
        .transpose(0, 3, 2, 1)
        .reshape(N, 128, NBLK * J)
    )

    in_maps = []
    for k in range(NCORES):
        n0 = NPER * k
        xts_k = np.ascontiguousarray(
            xt_all[n0 : n0 + NPER].reshape(NPER * 128, NBLK * J)
        )
        acts_k = np.ascontiguousarray(
            act[n0 : n0 + NPER].astype(act_np).reshape(NPER * C, W)
        )
        in_maps.append({"acts": acts_k, "xts": xts_k, "wm": wmat})
    return in_maps


def kernel(x, c, act):
    from concourse.bass_utils import run_bass_kernel_spmd

    x = np.ascontiguousarray(np.asarray(x), dtype=np.float32)
    c = np.ascontiguousarray(np.asarray(c), dtype=np.float32)
    act = np.ascontiguousarray(np.asarray(act), dtype=np.float32)
    assert x.shape == (N, D, W) and c.shape == (D, C) and act.shape == (N, C, W)

    in_maps = _stage(x, c, act)
    res = run_bass_kernel_spmd(_get_nc(), in_maps, core_ids=list(range(NCORES)))
    total = np.float32(0.0)
    for r in res.results:
        total = np.float32(total + np.sum(r["out"], dtype=np.float32))
    return np.maximum(np.float32(total), np.float32(1e-6))
